# revision 1
# baseline (speedup 1.0000x reference)
"""GAT layer (nn_GATLayer) Trainium2 Bass kernel.

Math (reference):
    h  = X @ W                                     # [N, D]
    s1 = h @ a[:D, 0] ; s2 = h @ a[D:, 0]          # [N]
    e  = exp(leaky_relu(s1[i] + s2[j], 0.2)) * (Adj != 0)
    out = (e / e.sum(axis=1, keepdims=True)) @ h

Kernel decomposition (per core, rows i in its block of 1024):
    z = s1[i] + s2[j]
    exp(lrelu(z)) = exp(0.2 z) * max(exp(0.8 z), 1)
    exp(a z) = exp(a s1[i]) * exp(a s2[j])   (rank-1)
  so with u2=exp(0.2 s1), v2=exp(0.2 s2), u3=exp(0.8 s1), v3=exp(0.8 s2):
    e_T[j, i] = (Adj_T[j, i] * u2[i] * v2[j]) * max(u3[i] * v3[j], 1)
  The (Adj_T * u2[i]) factor comes out of the PE by multiplying each
  128x128 Adj block with diag(u2) (a regular matmul that also transposes),
  v2[j] is a per-partition scalar fused into a DVE scalar_tensor_tensor,
  and the max() factor is one GpSimd tensor_scalar on a broadcast tile.
  Row sums ride along the main matmul as a ones-column appended to h.

Sharding: rows of X/Adj across 8 cores; W/a replicated; h block computed
locally then AllGathered through DRAM so every core has the full h.
"""

import sys
from contextlib import ExitStack

for _p in ("/opt/trn_rl_repo", "/root/.axon_site/_ro/trn_rl_repo"):
    if _p not in sys.path:
        sys.path.insert(0, _p)

import numpy as np

import concourse.bacc as bacc
import concourse.bass as bass
import concourse.mybir as mybir
from concourse import tile
from concourse.bass import ts
from concourse.bass_utils import run_bass_kernel_spmd
from concourse.masks import make_identity

F32 = mybir.dt.float32
BF16 = mybir.dt.bfloat16
F16 = mybir.dt.float16
I32 = mybir.dt.int32
AF = mybir.ActivationFunctionType
OP = mybir.AluOpType

N = 8192          # nodes
K = 512           # in dim
D = 64            # out dim
NCORES = 8
NB = N // NCORES  # 1024 rows per core
JC = N // 128     # 64 j-chunks
IC = NB // 128    # 8 i-chunks per core
ALPHA = 0.2
LN2 = 0.6931471805599453
ESCALE2_LN = -6 * LN2   # ln(2^-6) folded into v2's exp
ESCALE3_LN = -8 * LN2   # ln(2^-8) folded into v3's exp; q's max-const becomes 2^-8
QFLOOR = 2.0 ** -8      # scaled '1' in max(e^{0.8z}, 1)
# net uniform e scale 2^-14 cancels in the softmax normalization


def gat_kernel(tc, out_ap, x_ap, adj_ap, w_ap, a_ap, repeat=1, hoist_dma=False,
               ablate=None, delay_engine=None, delay_us=0):
    """Build the per-core program. All APs are DRAM access patterns:
    out [NB, D] f32, x [NB, K] f32, adj [NB, N] i32, w [K, D] f32, a [2D, 1] f32.
    delay_engine/delay_us: add a self-chained busy block on one engine
    (measurement probe: makes that engine the critical path)."""
    nc = tc.nc
    octx = ExitStack()

    constp = octx.enter_context(tc.tile_pool(name="const", bufs=1))
    dramp = octx.enter_context(tc.tile_pool(name="dram", bufs=1, space="DRAM"))

    if delay_engine:
        DW = 12000
        dwork = constp.tile([128, DW], F32)
        nc.vector.memset(dwork[:], 1.00001)
        if delay_engine == "dve":
            per_us = (58 + DW / 2) / 960
            for _ in range(max(1, round(delay_us / per_us))):
                nc.vector.tensor_scalar(dwork[:], dwork[:], 1.0, None, OP.mult)
        elif delay_engine == "act":
            per_us = (224 + DW) / 1200
            for _ in range(max(1, round(delay_us / per_us))):
                nc.scalar.copy(dwork[:], dwork[:])
        elif delay_engine == "pool":
            per_us = DW * 1.03 / 1200  # rough gpsimd line-rate guess
            for _ in range(max(1, round(delay_us / per_us))):
                nc.gpsimd.tensor_copy(dwork[:], dwork[:])

    # ---------------- constants ----------------
    eye = constp.tile([128, 128], F32)
    make_identity(nc, eye[:])
    ones_row = constp.tile([1, 128], F32)
    nc.vector.memset(ones_row[:], 1.0)

    # a broadcast across partitions: ab[:, 0:64] = a1, ab[:, 64:128] = a2
    a_row = constp.tile([1, 2 * D], F32)
    nc.sync.dma_start(a_row[:], a_ap.rearrange("d one -> one d"))

    with tc.tile_pool(name="pre_ps", bufs=1, space="PSUM") as pre_ps, \
         tc.tile_pool(name="pre_sb", bufs=3) as pre_sb:
        ab_ps = pre_ps.tile([128, 2 * D], F32)
        nc.tensor.matmul(ab_ps[:], lhsT=ones_row[:], rhs=a_row[:], start=True, stop=True)
        ab = constp.tile([128, 2 * D], F32)
        nc.vector.tensor_copy(ab[:], ab_ps[:])

        # ---------------- h = X @ W for own block ----------------
        wr = constp.tile([128, 4, D], F32)
        nc.sync.dma_start(wr[:], w_ap.rearrange("(kc p) d -> p kc d", p=128))

        h_shard = dramp.tile([NB, D], F32)
        s1_all = constp.tile([128, IC], F32)
        x3 = x_ap.rearrange("(t p) k -> p t k", p=128)
        junk1 = constp.tile([128, D], F32)
        for t in range(IC):
            xs = pre_sb.tile([128, K], F32, tag="xs")
            nc.sync.dma_start(xs[:], x3[:, t, :])
            xt_ps = pre_ps.tile([128, 4, 128], F32, tag="xt_ps")
            for kc in range(4):
                nc.tensor.transpose(xt_ps[:, kc, :], xs[:, ts(kc, 128)], eye[:])
            xt = pre_sb.tile([128, 4, 128], F32, tag="xt")
            nc.vector.tensor_copy(xt[:], xt_ps[:])
            h_ps = pre_ps.tile([128, D], F32, tag="h_ps")
            for kc in range(4):
                nc.tensor.matmul(h_ps[:], lhsT=xt[:, kc, :], rhs=wr[:, kc, :],
                                 start=(kc == 0), stop=(kc == 3))
            h_sb = pre_sb.tile([128, D], F32, tag="h_sb")
            nc.scalar.copy(h_sb[:], h_ps[:])
            nc.sync.dma_start(h_shard[ts(t, 128), :], h_sb[:])
            # s1 for own rows (pre-allgather, so indices stay core-local)
            nc.vector.scalar_tensor_tensor(junk1[:], h_sb[:], 1.0, ab[:, 0:D],
                                           OP.bypass, OP.mult,
                                           accum_out=s1_all[:, t:t + 1])

        # ---------------- AllGather h ----------------
        h_full = dramp.tile([N, D], F32)
        nc.gpsimd.collective_compute(
            "AllGather", OP.bypass,
            replica_groups=[list(range(NCORES))],
            ins=[h_shard.opt()],
            outs=[h_full.opt()],
        )

        # h_f32[p, c, :] = h[c*128+p, :]; h_aug is its bf16 copy + ones column
        h_f32 = constp.tile([128, JC, D], F32)
        nc.sync.dma_start(h_f32[:], h_full[:].rearrange("(c p) d -> p c d", p=128))
        h_aug = constp.tile([128, JC, D + 1], F16)
        nc.vector.memset(h_aug[:], 1.0)
        nc.vector.tensor_copy(h_aug[:, :, 0:D], h_f32[:])

        # s2 over all nodes
        s2_all = constp.tile([128, JC], F32)
        junk2 = constp.tile([128, D], F32)
        for c in range(JC):
            nc.vector.scalar_tensor_tensor(junk2[:], h_f32[:, c, :], 1.0,
                                           ab[:, D:2 * D], OP.bypass, OP.mult,
                                           accum_out=s2_all[:, c:c + 1])

        # exp factors
        esc2 = constp.tile([128, 1], F32)
        nc.vector.memset(esc2[:], ESCALE2_LN)
        esc3 = constp.tile([128, 1], F32)
        nc.vector.memset(esc3[:], ESCALE3_LN)
        v2 = constp.tile([128, JC], F32)
        nc.scalar.activation(v2[:], s2_all[:], AF.Exp, scale=ALPHA, bias=esc2[:])
        v3 = constp.tile([128, JC], F32)
        nc.scalar.activation(v3[:], s2_all[:], AF.Exp, scale=1.0 - ALPHA, bias=esc3[:])
        u2 = constp.tile([128, IC], F32)
        nc.scalar.activation(u2[:], s1_all[:], AF.Exp, scale=ALPHA)
        u3 = constp.tile([128, IC], F32)
        nc.scalar.activation(u3[:], s1_all[:], AF.Exp, scale=1.0 - ALPHA)

        # diag(u2[ic]) tiles for the transpose-and-scale matmuls. bf16 is safe:
        # u2[i] scales a whole attention row, so its rounding cancels in the
        # normalization; adj is 0/1 (exact in bf16).
        dg = constp.tile([128, IC, 128], F16)
        for ic in range(IC):
            nc.vector.tensor_scalar(dg[:, ic, :], eye[:], u2[:, ic:ic + 1], None, OP.mult)

        # u3 broadcast along free dim: u3_b[p, ic, q] = u3[q, ic] (= u3 of node ic*128+q)
        u3t_ps = pre_ps.tile([IC, 128], F32, tag="u3t_ps")
        nc.tensor.transpose(u3t_ps[:], u3[:], eye[:])
        u3t = pre_sb.tile([IC, 128], F32, tag="u3t")
        nc.vector.tensor_copy(u3t[:], u3t_ps[:])
        u3row = pre_sb.tile([1, NB], F32, tag="u3row")
        nc.sync.dma_start(u3row[:], u3t[:])  # flatten partitions into one row
        u3b_ps = pre_ps.tile([128, IC, 128], F32, tag="u3b_ps")
        for hh in range(2):
            nc.tensor.matmul(u3b_ps[:, 4 * hh:4 * hh + 4, :], lhsT=ones_row[:],
                             rhs=u3row[:, ts(hh, 512)], start=True, stop=True)
        u3_b = constp.tile([128, IC, 128], F16)
        nc.vector.tensor_copy(u3_b[:], u3b_ps[:])

    # ---------------- main loop over j-slabs (4 j-chunks each) ----------------
    JW = 512                      # j columns per DMA slab (2KB runs per (p, ic))
    NSLAB = N // JW               # 16
    JPS = JW // 128               # 4 j-chunks per slab
    adj3 = adj_ap.rearrange("(ic p) (s j) -> p ic s j", p=128, j=JW)

    out_ps_pool = octx.enter_context(tc.tile_pool(name="out_ps", bufs=1, space="PSUM"))
    outa_ps = out_ps_pool.tile([D + 1, 512], F32)
    outb_ps = out_ps_pool.tile([D + 1, 512], F32)

    with tc.tile_pool(name="adj_i", bufs=3) as adj_i_pool, \
         tc.tile_pool(name="adj_f", bufs=3) as adj_f_pool, \
         tc.tile_pool(name="p2", bufs=3, space="PSUM") as p2_pool, \
         tc.tile_pool(name="qe", bufs=4) as qe_pool:
        chk_dve = constp.tile([128, 1], F32)
        chk_act = constp.tile([128, 1], F32)

        hoisted = None
        if hoist_dma:
            hoisted = adj_i_pool.tile([128, IC, JW], I32, tag="adjh")
            nc.sync.dma_start(hoisted[:], adj3[:, :, 0, :])
        fx_adjb = fx_p2 = fx_q = fx_et = None
        if ablate == "conv":
            fx_adjb = adj_f_pool.tile([128, IC, JW], F16, tag="fxb")
            nc.vector.memset(fx_adjb[:], 1.0)
        if ablate == "diag":
            fx_p2 = out_ps_pool.tile([128, IC, 128], F32)
            for ic in range(IC):
                nc.tensor.matmul(fx_p2[:, ic, :], lhsT=dg[:, ic, :],
                                 rhs=dg[:, ic, :], start=True, stop=True)
        if ablate == "q":
            fx_q = constp.tile([128, IC, 128], F32)
            nc.vector.memset(fx_q[:], 1.0)
        if ablate == "dve":
            fx_et = constp.tile([128, IC, 128], F16)
            nc.vector.memset(fx_et[:], 1.0)
        for rep in range(repeat):
            for s in range(NSLAB):
                if hoist_dma:
                    adji = hoisted
                else:
                    adji = adj_i_pool.tile([128, IC, JW], I32, tag="adji")
                    nc.sync.dma_start(adji[:], adj3[:, :, s, :])
                if ablate == "conv":
                    adjb = fx_adjb
                    nc.vector.tensor_scalar(chk_dve[:], adji[:, 0, 0:1], 1.0,
                                            None, OP.mult)
                else:
                    adjb = adj_f_pool.tile([128, IC, JW], F16, tag="adjb")
                    nc.scalar.copy(adjb[:], adji[:])
                for js in range(JPS):
                    jc = s * JPS + js
                    if ablate == "diag":
                        p2_ps = fx_p2
                        nc.vector.tensor_scalar(chk_dve[:], adjb[:, 0, 0:1], 1.0,
                                                None, OP.mult)
                    else:
                        p2_ps = p2_pool.tile([128, IC, 128], F32, tag="p2")
                        for ic in range(IC):
                            # p2[:, ic, q] over parts j: sum_i adj[i,j]*diag_u2[i,q]
                            nc.tensor.matmul(p2_ps[:, ic, :],
                                             lhsT=adjb[:, ic, ts(js, 128)],
                                             rhs=dg[:, ic, :], start=True, stop=True)

                    if ablate == "q":
                        q = fx_q
                    else:
                        q = qe_pool.tile([128, IC, 128], F16, tag="q")
                        nc.vector.tensor_scalar(q[:], u3_b[:], v3[:, jc:jc + 1],
                                                QFLOOR, OP.mult, OP.max)

                    if ablate == "dve":
                        e_t = fx_et
                        nc.scalar.copy(chk_act[:], p2_ps[:, 0, 0:1])
                        nc.gpsimd.tensor_scalar(q[:, 0, 0:1], q[:, 0, 0:1],
                                                1.0, None, OP.mult)
                    else:
                        e_t = qe_pool.tile([128, IC, 128], F16, tag="e_t")
                        nc.vector.scalar_tensor_tensor(e_t[:], p2_ps[:],
                                                       v2[:, jc:jc + 1],
                                                       q[:], OP.mult, OP.mult)

                    first = (jc == 0) and (rep == 0)
                    last = (jc == JC - 1) and (rep == repeat - 1)
                    if ablate == "main" and not (first or last):
                        nc.vector.tensor_scalar(chk_dve[:], e_t[:, 0, 0:1], 1.0,
                                                None, OP.mult)
                    else:
                        nc.tensor.matmul(outa_ps[:], lhsT=h_aug[:, jc, :],
                                         rhs=e_t[:, 0:4, :], start=first, stop=last)
                        nc.tensor.matmul(outb_ps[:], lhsT=h_aug[:, jc, :],
                                         rhs=e_t[:, 4:8, :], start=first, stop=last)

    # ---------------- normalize + transpose back + store ----------------
    with tc.tile_pool(name="post_sb", bufs=2) as post_sb, \
         tc.tile_pool(name="post_ps", bufs=2, space="PSUM") as post_ps:
        for half, o_ps in enumerate((outa_ps, outb_ps)):
            osb = post_sb.tile([D + 1, 512], F32, tag="osb")
            nc.scalar.copy(osb[:], o_ps[:])
            for b in range(4):
                o2_ps = post_ps.tile([128, D + 1], F32, tag="o2")
                nc.tensor.transpose(o2_ps[:], osb[:, ts(b, 128)], eye[0:D + 1, 0:D + 1])
                rcp = post_sb.tile([128, 1], F32, tag="rcp")
                nc.vector.reciprocal(rcp[:], o2_ps[:, D:D + 1])
                fin = post_sb.tile([128, D], F32, tag="fin")
                nc.vector.tensor_scalar(fin[:], o2_ps[:, 0:D], rcp[:], None, OP.mult)
                nc.sync.dma_start(out_ap[bass.ds(half * 512 + b * 128, 128), :], fin[:])

    octx.close()


_BUILT = {}


def _build(repeat=1, hoist_dma=False, ablate=None, delay_engine=None, delay_us=0):
    key = (repeat, hoist_dma, ablate, delay_engine, delay_us)
    if key in _BUILT:
        return _BUILT[key]
    nc = bacc.Bacc("TRN2", target_bir_lowering=False, debug=False,
                   enable_asserts=False, num_devices=NCORES)
    x = nc.dram_tensor("X_blk", [NB, K], F32, kind="ExternalInput")
    adj = nc.dram_tensor("Adj_blk", [NB, N], I32, kind="ExternalInput")
    w = nc.dram_tensor("W", [K, D], F32, kind="ExternalInput")
    a = nc.dram_tensor("a", [2 * D, 1], F32, kind="ExternalInput")
    out = nc.dram_tensor("out", [NB, D], F32, kind="ExternalOutput")
    with tile.TileContext(nc) as tc:
        gat_kernel(tc, out.ap(), x.ap(), adj.ap(), w.ap(), a.ap(), repeat=repeat,
                   hoist_dma=hoist_dma, ablate=ablate,
                   delay_engine=delay_engine, delay_us=delay_us)
    nc.compile()
    _BUILT[key] = nc
    return nc


def kernel(X, Adj, W, a, _trace=False):
    X = np.ascontiguousarray(np.asarray(X, dtype=np.float32))
    Adj = np.ascontiguousarray(np.asarray(Adj, dtype=np.int32))
    W = np.ascontiguousarray(np.asarray(W, dtype=np.float32))
    a = np.ascontiguousarray(np.asarray(a, dtype=np.float32))

    nc = _build()
    in_maps = [
        {
            "X_blk": X[c * NB:(c + 1) * NB],
            "Adj_blk": Adj[c * NB:(c + 1) * NB],
            "W": W,
            "a": a,
        }
        for c in range(NCORES)
    ]
    res = run_bass_kernel_spmd(nc, in_maps, core_ids=list(range(NCORES)),
                               trace=_trace)
    out = np.concatenate([res.results[c]["out"] for c in range(NCORES)], axis=0)
    if _trace:
        kernel.last_results = res
    return out

